# revision 20
# baseline (speedup 1.0000x reference)
"""Trainium2 Bass kernel for BiDAF-style bidirectional attention.

Reference computation (per batch element n; M=1 folded away):
    s[i,j]  = h[i].w_h + u[j].w_u + (h[i]*u[j]).w_hu + b      [JX, JQ]
    a_u     = softmax_j(s);     u_a[i] = sum_j a_u[i,j] u[j]   (c2q)
    a_h     = softmax_i(max_j s);  h_a = sum_i a_h[i] h[i]     (q2c)
    out     = concat(h, u_a, h*u_a, h*h_a)                     [JX, 4D]

Sharding: data-parallel over batch N=8, one NeuronCore per batch element.
alpha_b drops out (both softmaxes are shift-invariant); accepted but unused.
The first output slice is h verbatim, so the device computes and stores only
[u_a | h*u_a | h*h_a] ([JX, 3D]); the host writes the h slice during unshard.

Per-core dataflow (i = context position, j = query position, d = feature):
  - h and u dram tensors are declared float32r (same bit layout as f32) so
    matmuls and transposes touching them run in the fast replicated mode
    without any on-chip recast.
  - scores are computed TRANSPOSED in 4 blocks of 256 i-columns:
    s0T[j,i] = sum_d uwT[d,j] hT[d,i] accumulated over 4 d-chunks, with
    h.w_h folded in via a K=1 matmul (ones_row outer hwh_row) and u.w_u as
    the per-partition bias of the ScalarE Exp evict: ET = exp(sT).
  - per block: PE re-transposes ET (2 tiles in one PSUM bank); one 3D DVE
    reduce pair gives row maxes/sums; u_a = (ET_tile^T @ u) scaled by
    1/rowsum (ScalarE evict); o3 = h*u_a on DVE; one [128 x 4KB] store per
    tile.  q2c weights accumulate per tile into a single PSUM group.
  - q2c tail: global sum via ones matmul, broadcast h_a with a K=1 matmul,
    o4 = h*h_a on GpSimd/DVE, stored as 2-tile pairs.
  - blocks are pipelined: block 0's store issues while h tiles for later
    blocks are still loading, keeping the 16 DMA engines streaming.
A short PE dummy-matmul burst at kernel start lifts the HAM clock gate
while the first h tiles are still in flight.
"""

import numpy as np

N_B, M_B, JX, JQ, D = 8, 1, 1024, 128, 512
P = 128
NT = JX // P    # 8 i-tiles
KC = D // P     # 4 d-chunks
TPB = 2         # tiles per score block
NB = NT // TPB  # 4 blocks
IB = TPB * P    # 256 i-columns per block

_CACHE = {}


def _build_program():
    from contextlib import ExitStack

    import concourse.bass as bass
    import concourse.tile as tile
    from concourse import bacc, mybir
    from concourse.masks import make_identity

    f32 = mybir.dt.float32
    f32r = mybir.dt.float32r
    EXP = mybir.ActivationFunctionType.Exp
    AX = mybir.AxisListType.X
    ds = bass.ds

    nc = bacc.Bacc("TRN2", target_bir_lowering=False, debug=False, num_devices=8)
    h_d = nc.dram_tensor("h", [JX, D], f32r, kind="ExternalInput").ap()
    u_d = nc.dram_tensor("u", [JQ, D], f32r, kind="ExternalInput").ap()
    aw_d = nc.dram_tensor("alpha_w", [3 * D], f32r, kind="ExternalInput").ap()
    out_d = nc.dram_tensor("out", [JX, 3 * D], f32, kind="ExternalOutput").ap()

    with tile.TileContext(nc) as tc, ExitStack() as ctx:
        consts = ctx.enter_context(tc.tile_pool(name="consts", bufs=1))
        stage = ctx.enter_context(tc.tile_pool(name="stage", bufs=6))
        # PSUM budget (8 banks): tp=2, s0=2 (shared with warmup), ua=2,
        # acc=1, hap=1
        ps = ctx.enter_context(tc.tile_pool(name="ps", bufs=2, space="PSUM"))

        # ---- input DMAs: small tensors first (a tiny DMA issued late lands
        # after all earlier bulk on the shared engine rings), then h tiles.
        h_all = consts.tile([P, NT * D], f32r)  # tile t: h[t*128+p, d]
        def h_load(eng, t0, nt):
            src = h_d[ds(t0 * P, nt * P), :].rearrange("(t p) d -> p t d", p=P)
            dst = h_all[:, ds(t0 * D, nt * D)].rearrange("p (t d) -> p t d", d=D)
            eng.dma_start(dst, src)
        h_load(nc.sync, 0, 1)
        w_cm = consts.tile([12, P], f32r)  # alpha_w chunk-major (contiguous)
        nc.sync.dma_start(w_cm[:], aw_d.rearrange("(c p) -> c p", p=P))
        w_flat = consts.tile([1, 3 * D], f32r)  # alpha_w on one partition
        nc.sync.dma_start(w_flat[:], aw_d.rearrange("(o w) -> o w", o=1))
        u_sb = consts.tile([JQ, D], f32r)
        nc.sync.dma_start(u_sb[:], u_d[:])
        h_load(nc.sync, 1, 1)
        h_load(nc.sync, 2, 2)
        h_load(nc.sync, 4, 2)
        h_load(nc.sync, 6, 2)

        # ---- PE warmup: 512-wide dummy f32r matmuls from the first free
        # cycle until all loads have landed, so the HAM clock ramp
        # (1.2 -> 2.4 GHz after ~5.7us of continuous PE activity) completes
        # right as the real dependency chain gets going.
        warm_f = consts.tile([P, D], f32)
        nc.vector.memset(warm_f[:], 0.25)
        warm = consts.tile([P, D], f32r)
        nc.vector.tensor_copy(warm[:], warm_f[:])
        wp = ps.tile([P, D], f32, tag="s0")
        def warm_mm(n):
            for _ in range(n):
                nc.tensor.matmul(
                    wp[:], warm[:, ds(0, P)], warm[:], start=True, stop=True,
                )
        warm_mm(2)
        # dummy Exp: forces the 1.3us ACT_TABLE_LOAD to the front of the
        # scalar queue instead of the middle of the score critical path
        warm_e = consts.tile([1, 1], f32)
        nc.scalar.activation(warm_e[:], warm_f[ds(0, 1), ds(0, 1)], EXP)

        # ---- constants ----
        ident = consts.tile([P, P], f32)
        make_identity(nc, ident[:])
        ident_r = consts.tile([P, P], f32r)
        nc.vector.tensor_copy(ident_r[:], ident[:])
        ones_col = consts.tile([P, 1], f32)
        nc.vector.memset(ones_col[:], 1.0)
        ones_fr = consts.tile([1, P], f32)
        nc.vector.memset(ones_fr[:], 1.0)
        ones_row_r = consts.tile([1, P], f32r)
        nc.vector.tensor_copy(ones_row_r[:], ones_fr[:])

        hT_all = consts.tile([P, KC * JX], f32r)  # chunk k: hT[k*128+p, i]
        hT3 = hT_all[:].rearrange("p (k x) -> p k x", k=KC)
        hwh_row = consts.tile([1, JX], f32r)      # h.w_h as a row over i
        ET = consts.tile([JQ, JX], f32r)          # exp(s0T + uwu[j] + hwh[i])
        m_exp_r = consts.tile([P, NT], f32r)      # per i-tile: max_j ET
        z_rec = consts.tile([P, NT], f32)         # per i-tile: 1/sum_j ET
        hap = ps.tile([1, D], f32, tag="hap", bufs=1)

        def transpose_tile(t, evict):
            tp = ps.tile([P, KC * P], f32r, tag="tp")
            for k in range(KC):
                nc.tensor.transpose(
                    tp[:, ds(k * P, P)], h_all[:, ds(t * D + k * P, P)],
                    ident_r[:],
                )
            evict(hT3[:, :, ds(t * P, P)], tp[:].rearrange("p (k x) -> p k x", k=KC))

        def prep_weights():
            # w_cols[p, c] = alpha_w[c*128+p] via one PE transpose of w_cm;
            # broadcast w_u / w_hu across partitions with K=1 matmuls so
            # uw = u * w_hu and uwu = sum_d u[j,d] w_u[d] are single DVE ops
            wtp = ps.tile([P, 12], f32r, tag="acc", bufs=1)
            nc.tensor.transpose(wtp[:], w_cm[:], ident_r[ds(0, 12), ds(0, 12)])
            w_cols_r = consts.tile([P, 12], f32r)
            nc.vector.tensor_copy(w_cols_r[:], wtp[:])
            wb_u = ps.tile([JQ, D], f32, tag="hap", bufs=1)
            nc.tensor.matmul(
                wb_u[:], ones_row_r[:, ds(0, JQ)], w_flat[:, ds(D, D)],
                start=True, stop=True,
            )
            wb_hu = ps.tile([JQ, D], f32, tag="acc", bufs=1)
            nc.tensor.matmul(
                wb_hu[:], ones_row_r[:, ds(0, JQ)], w_flat[:, ds(2 * D, D)],
                start=True, stop=True,
            )
            uwu = consts.tile([JQ, 1], f32)
            uw_scr = stage.tile([JQ, D], f32, tag="stg")
            nc.vector.scalar_tensor_tensor(
                uw_scr[:], u_sb[:], 1.0, wb_u[:],
                op0=mybir.AluOpType.mult, op1=mybir.AluOpType.mult,
                accum_out=uwu[:],
            )
            uw = consts.tile([JQ, D], f32r)
            nc.vector.tensor_mul(uw[:], u_sb[:], wb_hu[:])
            return w_cols_r, uw, uwu

        def prep_uwT(uw):
            pt = ps.tile([P, KC * P], f32r, tag="tp")
            for k in range(KC):
                nc.tensor.transpose(
                    pt[:, ds(k * P, P)], uw[:, ds(k * P, P)], ident_r[:]
                )
            uwT = consts.tile([P, KC * JQ], f32r)
            nc.scalar.copy(uwT[:], pt[:])
            return uwT

        def block_scores(t0, nt, w_cols_r, uwT):
            ib = nt * P
            blk = ds(t0 * P, ib)
            hp = ps.tile([1, ib], f32, tag="acc", bufs=1)
            for k in range(KC):
                nc.tensor.matmul(
                    hp[:], w_cols_r[:, ds(k, 1)], hT_all[:, ds(k * JX + t0 * P, ib)],
                    start=(k == 0), stop=(k == KC - 1),
                )
            nc.scalar.copy(hwh_row[:, blk], hp[:])
            sp = ps.tile([JQ, ib], f32, tag="s0")
            for k in range(KC):
                nc.tensor.matmul(
                    sp[:], uwT[:, ds(k * JQ, JQ)], hT_all[:, ds(k * JX + t0 * P, ib)],
                    start=(k == 0), stop=False,
                )
            nc.tensor.matmul(
                sp[:], ones_row_r[:], hwh_row[:, blk], start=False, stop=True
            )
            return sp

        def block_softmax_c2q(t0, nt, sp, uwu, latency=False, tail_hook=None):
            blk = ds(t0 * P, nt * P)
            nc.scalar.activation(ET[:, blk], sp[:], EXP, bias=uwu[:])
            et = ps.tile([P, nt * P], f32r, tag="tp")
            for q in range(nt):
                t = t0 + q
                nc.tensor.transpose(
                    et[:, ds(q * P, P)], ET[:, ds(t * P, P)], ident_r[:]
                )
            et3 = et[:].rearrange("p (q x) -> p q x", q=nt)
            nc.vector.reduce_max(m_exp_r[:, ds(t0, nt)], et3, axis=AX)
            zsum = stage.tile([P, nt], f32, tag="zs")
            nc.vector.reduce_sum(zsum[:], et3, axis=AX)
            nc.vector.reciprocal(z_rec[:, ds(t0, nt)], zsum[:])
            ups = []
            for q in range(nt):
                t = t0 + q
                up = ps.tile([P, D], f32, tag="ua")
                nc.tensor.matmul(
                    up[:], ET[:, ds(t * P, P)], u_sb[:], start=True, stop=True
                )
                ups.append(up)
            for q in range(nt):
                t = t0 + q
                # q2c accumulation (single PSUM group spanning all blocks)
                nc.tensor.matmul(
                    hap[:], m_exp_r[:, ds(t, 1)], h_all[:, ds(t * D, D)],
                    start=(t == 0), stop=(t == NT - 1),
                    skip_group_check=True,
                )
            if tail_hook is not None:
                tail_hook()
            for q in range(nt):
                t = t0 + q
                up = ups[q]
                stg = stage.tile([P, 2 * D], f32, tag="stg")
                if latency:
                    # latency-critical early blocks: o2/o3 in parallel from
                    # PSUM on the two engines that can read it
                    nc.scalar.mul(stg[:, ds(0, D)], up[:], z_rec[:, ds(t, 1)])
                    nc.vector.scalar_tensor_tensor(
                        stg[:, ds(D, D)], up[:], z_rec[:, ds(t, 1)],
                        h_all[:, ds(t * D, D)],
                        op0=mybir.AluOpType.mult, op1=mybir.AluOpType.mult,
                    )
                else:
                    # throughput phase: o2 alternates ScalarE/DVE, GpSimd
                    # builds o3 from SBUF
                    if t % 2 == 0:
                        nc.scalar.mul(stg[:, ds(0, D)], up[:], z_rec[:, ds(t, 1)])
                    else:
                        nc.vector.tensor_scalar_mul(
                            stg[:, ds(0, D)], up[:], z_rec[:, ds(t, 1)]
                        )
                    nc.gpsimd.tensor_mul(
                        stg[:, ds(D, D)], stg[:, ds(0, D)],
                        h_all[:, ds(t * D, D)],
                    )
                nc.sync.dma_start(out_d[ds(t * P, P), ds(0, 2 * D)], stg[:])

        # Software-pipelined emission: warmup leads; block sizes ramp
        # [1,1,2,2,2] so the first store issues as early as possible; the
        # next block's transposes slot between a block's score matmuls and
        # its softmax tail to hide Exp/reduce latency.
        dve = nc.vector.tensor_copy
        sca = nc.scalar.copy
        BL = [(0, 1), (1, 1), (2, 2), (4, 2), (6, 2)]
        evs = {0: dve, 1: dve, 2: sca, 3: dve, 4: sca, 5: dve, 6: sca, 7: dve}

        mrow = consts.tile([P, 1], f32)
        rzq = consts.tile([1, 1], f32)
        ha_sum = consts.tile([1, D], f32)
        ha_row = consts.tile([1, D], f32r)
        zqp = [None]
        def q2c_tail():
            # emitted right after the final block's reduces: normalize and
            # broadcast h_a while that block's outputs are still in flight
            nc.vector.reduce_sum(mrow[:], m_exp_r[:], axis=AX)
            zq_ps = ps.tile([1, 1], f32, tag="acc", bufs=1)
            nc.tensor.matmul(zq_ps[:], mrow[:], ones_col[:], start=True, stop=True)
            nc.vector.reciprocal(rzq[:], zq_ps[:])
            nc.vector.tensor_copy(ha_sum[:], hap[:])
            nc.scalar.mul(ha_row[:], ha_sum[:], rzq[:])

        w_cols_r, uw, uwu = prep_weights()
        transpose_tile(0, dve)
        warm_mm(1)
        uwT = prep_uwT(uw)
        warm_mm(1)
        sp_prev = block_scores(0, 1, w_cols_r, uwT)
        warm_mm(1)
        for i in range(1, len(BL)):
            t0p, ntp = BL[i - 1]
            t0n, ntn = BL[i]
            for t in range(t0n, t0n + ntn):
                transpose_tile(t, evs[t])
            block_softmax_c2q(t0p, ntp, sp_prev, uwu, latency=(i <= 2))
            sp_prev = block_scores(t0n, ntn, w_cols_r, uwT)
        t0l, ntl = BL[-1]
        block_softmax_c2q(t0l, ntl, sp_prev, uwu, tail_hook=q2c_tail)

        # ---- o4 = h * h_a ----
        bc = ps.tile([P, D], f32, tag="acc", bufs=1)
        nc.tensor.matmul(bc[:], ones_row_r[:], ha_row[:], start=True, stop=True)
        bc_sb = consts.tile([P, D], f32)  # gpsimd cannot read PSUM
        nc.scalar.copy(bc_sb[:], bc[:])

        for pair in range(NT // 2):
            t0 = 2 * pair
            o4 = stage.tile([P, 2 * D], f32, tag="o4")
            nc.gpsimd.tensor_mul(o4[:, ds(0, D)], h_all[:, ds(t0 * D, D)], bc_sb[:])
            nc.vector.tensor_mul(o4[:, ds(D, D)], h_all[:, ds((t0 + 1) * D, D)], bc[:])
            eng = nc.scalar if pair % 2 == 0 else nc.sync
            eng.dma_start(
                out_d[ds(t0 * P, 2 * P), ds(2 * D, D)].rearrange(
                    "(t p) d -> p t d", p=P
                ),
                o4[:].rearrange("p (t d) -> p t d", d=D),
            )

    nc.compile()
    return nc


def _get_nc():
    if "nc" not in _CACHE:
        _CACHE["nc"] = _build_program()
    return _CACHE["nc"]


def _assemble(h, results):
    """Host-side unshard: column 0 of the output is h verbatim."""
    out = np.empty((N_B, JX, 4 * D), dtype=np.float32)
    out[:, :, :D] = h
    for n in range(N_B):
        out[n, :, D:] = results[n]["out"]
    return out.reshape(N_B, M_B, JX, 4 * D)


def _ensure_axon_hooks_stub():
    # concourse imports antenv.axon_hooks when tracing is requested via env;
    # provide a no-op stub if the image lacks it so runs degrade gracefully.
    import sys
    import types

    try:
        import antenv.axon_hooks  # noqa: F401
    except ImportError:
        mod = types.ModuleType("antenv.axon_hooks")
        _hook = [None]
        mod.set_axon_ntff_profile_hook = lambda hook: _hook.__setitem__(0, hook)
        mod.get_axon_ntff_profile_hook = lambda: _hook[0]
        sys.modules["antenv.axon_hooks"] = mod


def kernel(h, u, alpha_w, alpha_b=None, **_unused):
    _ensure_axon_hooks_stub()
    from concourse.bass_utils import run_bass_kernel_spmd

    h = np.ascontiguousarray(np.asarray(h, dtype=np.float32)).reshape(N_B, JX, D)
    u = np.ascontiguousarray(np.asarray(u, dtype=np.float32)).reshape(N_B, JQ, D)
    alpha_w = np.ascontiguousarray(np.asarray(alpha_w, dtype=np.float32)).reshape(3 * D)

    nc = _get_nc()
    in_maps = [
        {"h": h[n], "u": u[n], "alpha_w": alpha_w} for n in range(N_B)
    ]
    res = run_bass_kernel_spmd(nc, in_maps, core_ids=list(range(N_B)))
    return _assemble(h, res.results)


# revision 21
# speedup vs baseline: 1.2541x; 1.2541x over previous
"""Trainium2 Bass kernel for BiDAF-style bidirectional attention.

Reference computation (per batch element n; M=1 folded away):
    s[i,j]  = h[i].w_h + u[j].w_u + (h[i]*u[j]).w_hu + b      [JX, JQ]
    a_u     = softmax_j(s);     u_a[i] = sum_j a_u[i,j] u[j]   (c2q)
    a_h     = softmax_i(max_j s);  h_a = sum_i a_h[i] h[i]     (q2c)
    out     = concat(h, u_a, h*u_a, h*h_a)                     [JX, 4D]

Sharding: data-parallel over batch N=8, one NeuronCore per batch element.
alpha_b drops out (both softmaxes are shift-invariant); accepted but unused.
The first output slice is h verbatim, so the device computes and stores only
[u_a | h*u_a | h*h_a] ([JX, 3D]); the host writes the h slice during unshard.

Per-core dataflow (i = context position, j = query position, d = feature):
  - h and u dram tensors are declared float32r (same bit layout as f32) so
    matmuls and transposes touching them run in the fast replicated mode
    without any on-chip recast.
  - scores are computed TRANSPOSED in 4 blocks of 256 i-columns:
    s0T[j,i] = sum_d uwT[d,j] hT[d,i] accumulated over 4 d-chunks, with
    h.w_h folded in via a K=1 matmul (ones_row outer hwh_row) and u.w_u as
    the per-partition bias of the ScalarE Exp evict: ET = exp(sT).
  - per block: PE re-transposes ET (2 tiles in one PSUM bank); one 3D DVE
    reduce pair gives row maxes/sums; u_a = (ET_tile^T @ u) scaled by
    1/rowsum (ScalarE evict); o3 = h*u_a on DVE; one [128 x 4KB] store per
    tile.  q2c weights accumulate per tile into a single PSUM group.
  - q2c tail: global sum via ones matmul, broadcast h_a with a K=1 matmul,
    o4 = h*h_a on GpSimd/DVE, stored as 2-tile pairs.
  - blocks are pipelined: block 0's store issues while h tiles for later
    blocks are still loading, keeping the 16 DMA engines streaming.
A short PE dummy-matmul burst at kernel start lifts the HAM clock gate
while the first h tiles are still in flight.
"""

import numpy as np

N_B, M_B, JX, JQ, D = 8, 1, 1024, 128, 512
P = 128
NT = JX // P    # 8 i-tiles
KC = D // P     # 4 d-chunks
TPB = 2         # tiles per score block
NB = NT // TPB  # 4 blocks
IB = TPB * P    # 256 i-columns per block

_CACHE = {}


def _build_program():
    from contextlib import ExitStack

    import concourse.bass as bass
    import concourse.tile as tile
    from concourse import bacc, mybir
    from concourse.masks import make_identity

    f32 = mybir.dt.float32
    f32r = mybir.dt.float32r
    EXP = mybir.ActivationFunctionType.Exp
    AX = mybir.AxisListType.X
    ds = bass.ds

    nc = bacc.Bacc("TRN2", target_bir_lowering=False, debug=False, num_devices=8)
    h_d = nc.dram_tensor("h", [JX, D], f32r, kind="ExternalInput").ap()
    u_d = nc.dram_tensor("u", [JQ, D], f32r, kind="ExternalInput").ap()
    aw_d = nc.dram_tensor("alpha_w", [3 * D], f32r, kind="ExternalInput").ap()
    out_d = nc.dram_tensor("out", [JX, 3 * D], f32, kind="ExternalOutput").ap()

    with tile.TileContext(nc) as tc, ExitStack() as ctx:
        consts = ctx.enter_context(tc.tile_pool(name="consts", bufs=1))
        stage = ctx.enter_context(tc.tile_pool(name="stage", bufs=6))
        # PSUM budget (8 banks): tp=2, s0=2 (shared with warmup), ua=2,
        # acc=1, hap=1
        ps = ctx.enter_context(tc.tile_pool(name="ps", bufs=2, space="PSUM"))

        # ---- input DMAs: small tensors first (a tiny DMA issued late lands
        # after all earlier bulk on the shared engine rings), then h tiles.
        h_all = consts.tile([P, NT * D], f32r)  # tile t: h[t*128+p, d]
        def h_load(eng, t0, nt):
            src = h_d[ds(t0 * P, nt * P), :].rearrange("(t p) d -> p t d", p=P)
            dst = h_all[:, ds(t0 * D, nt * D)].rearrange("p (t d) -> p t d", d=D)
            eng.dma_start(dst, src)
        h_load(nc.sync, 0, 1)
        w_cm = consts.tile([12, P], f32r)  # alpha_w chunk-major (contiguous)
        nc.sync.dma_start(w_cm[:], aw_d.rearrange("(c p) -> c p", p=P))
        w_flat = consts.tile([1, 3 * D], f32r)  # alpha_w on one partition
        nc.sync.dma_start(w_flat[:], aw_d.rearrange("(o w) -> o w", o=1))
        u_sb = consts.tile([JQ, D], f32r)
        nc.sync.dma_start(u_sb[:], u_d[:])
        h_load(nc.sync, 1, 1)
        h_load(nc.sync, 2, 2)
        h_load(nc.sync, 4, 2)
        h_load(nc.sync, 6, 2)

        # ---- PE warmup: 512-wide dummy f32r matmuls from the first free
        # cycle until all loads have landed, so the HAM clock ramp
        # (1.2 -> 2.4 GHz after ~5.7us of continuous PE activity) completes
        # right as the real dependency chain gets going.
        warm_f = consts.tile([P, D], f32)
        nc.vector.memset(warm_f[:], 0.25)
        warm = consts.tile([P, D], f32r)
        nc.vector.tensor_copy(warm[:], warm_f[:])
        wp = ps.tile([P, D], f32, tag="s0")
        def warm_mm(n):
            for _ in range(n):
                nc.tensor.matmul(
                    wp[:], warm[:, ds(0, P)], warm[:], start=True, stop=True,
                )
        warm_mm(2)
        # dummy Exp: forces the 1.3us ACT_TABLE_LOAD to the front of the
        # scalar queue instead of the middle of the score critical path
        warm_e = consts.tile([1, 1], f32)
        nc.scalar.activation(warm_e[:], warm_f[ds(0, 1), ds(0, 1)], EXP)

        # ---- constants ----
        ident = consts.tile([P, P], f32)
        make_identity(nc, ident[:])
        ident_r = consts.tile([P, P], f32r)
        nc.vector.tensor_copy(ident_r[:], ident[:])
        ones_col = consts.tile([P, 1], f32)
        nc.vector.memset(ones_col[:], 1.0)
        ones_fr = consts.tile([1, P], f32)
        nc.vector.memset(ones_fr[:], 1.0)
        ones_row_r = consts.tile([1, P], f32r)
        nc.vector.tensor_copy(ones_row_r[:], ones_fr[:])

        hT_all = consts.tile([P, KC * JX], f32r)  # chunk k: hT[k*128+p, i]
        hT3 = hT_all[:].rearrange("p (k x) -> p k x", k=KC)
        hwh_row = consts.tile([1, JX], f32r)      # h.w_h as a row over i
        ET = consts.tile([JQ, JX], f32r)          # exp(s0T + uwu[j] + hwh[i])
        m_exp_r = consts.tile([P, NT], f32r)      # per i-tile: max_j ET
        z_rec = consts.tile([P, NT], f32)         # per i-tile: 1/sum_j ET
        hap = ps.tile([1, D], f32, tag="hap", bufs=1)

        def transpose_tile(t, evict):
            tp = ps.tile([P, KC * P], f32r, tag="tp")
            for k in range(KC):
                nc.tensor.transpose(
                    tp[:, ds(k * P, P)], h_all[:, ds(t * D + k * P, P)],
                    ident_r[:],
                )
            evict(hT3[:, :, ds(t * P, P)], tp[:].rearrange("p (k x) -> p k x", k=KC))

        def prep_weights():
            # w_cols[p, c] = alpha_w[c*128+p] via one PE transpose of w_cm;
            # broadcast w_u / w_hu across partitions with K=1 matmuls so
            # uw = u * w_hu and uwu = sum_d u[j,d] w_u[d] are single DVE ops
            wtp = ps.tile([P, 12], f32r, tag="acc", bufs=1)
            nc.tensor.transpose(wtp[:], w_cm[:], ident_r[ds(0, 12), ds(0, 12)])
            w_cols_r = consts.tile([P, 12], f32r)
            nc.vector.tensor_copy(w_cols_r[:], wtp[:])
            wb_u = ps.tile([JQ, D], f32, tag="hap", bufs=1)
            nc.tensor.matmul(
                wb_u[:], ones_row_r[:, ds(0, JQ)], w_flat[:, ds(D, D)],
                start=True, stop=True,
            )
            wb_hu = ps.tile([JQ, D], f32, tag="acc", bufs=1)
            nc.tensor.matmul(
                wb_hu[:], ones_row_r[:, ds(0, JQ)], w_flat[:, ds(2 * D, D)],
                start=True, stop=True,
            )
            uwu = consts.tile([JQ, 1], f32)
            uw_scr = stage.tile([JQ, D], f32, tag="stg")
            nc.vector.scalar_tensor_tensor(
                uw_scr[:], u_sb[:], 1.0, wb_u[:],
                op0=mybir.AluOpType.mult, op1=mybir.AluOpType.mult,
                accum_out=uwu[:],
            )
            uw = consts.tile([JQ, D], f32r)
            nc.vector.tensor_mul(uw[:], u_sb[:], wb_hu[:])
            return w_cols_r, uw, uwu

        def prep_uwT(uw):
            pt = ps.tile([P, KC * P], f32r, tag="tp")
            for k in range(KC):
                nc.tensor.transpose(
                    pt[:, ds(k * P, P)], uw[:, ds(k * P, P)], ident_r[:]
                )
            uwT = consts.tile([P, KC * JQ], f32r)
            nc.scalar.copy(uwT[:], pt[:])
            return uwT

        def block_scores(t0, nt, w_cols_r, uwT):
            ib = nt * P
            blk = ds(t0 * P, ib)
            hp = ps.tile([1, ib], f32, tag="acc", bufs=1)
            for k in range(KC):
                nc.tensor.matmul(
                    hp[:], w_cols_r[:, ds(k, 1)], hT_all[:, ds(k * JX + t0 * P, ib)],
                    start=(k == 0), stop=(k == KC - 1),
                )
            nc.scalar.copy(hwh_row[:, blk], hp[:])
            sp = ps.tile([JQ, ib], f32, tag="s0")
            for k in range(KC):
                nc.tensor.matmul(
                    sp[:], uwT[:, ds(k * JQ, JQ)], hT_all[:, ds(k * JX + t0 * P, ib)],
                    start=(k == 0), stop=False,
                )
            nc.tensor.matmul(
                sp[:], ones_row_r[:], hwh_row[:, blk], start=False, stop=True
            )
            return sp

        def block_softmax_c2q(t0, nt, sp, uwu, latency=False, tail_hook=None):
            blk = ds(t0 * P, nt * P)
            nc.scalar.activation(ET[:, blk], sp[:], EXP, bias=uwu[:])
            et = ps.tile([P, nt * P], f32r, tag="tp")
            for q in range(nt):
                t = t0 + q
                nc.tensor.transpose(
                    et[:, ds(q * P, P)], ET[:, ds(t * P, P)], ident_r[:]
                )
            et3 = et[:].rearrange("p (q x) -> p q x", q=nt)
            nc.vector.reduce_max(m_exp_r[:, ds(t0, nt)], et3, axis=AX)
            zsum = stage.tile([P, nt], f32, tag="zs")
            nc.vector.reduce_sum(zsum[:], et3, axis=AX)
            nc.vector.reciprocal(z_rec[:, ds(t0, nt)], zsum[:])
            ups = []
            for q in range(nt):
                t = t0 + q
                up = ps.tile([P, D], f32, tag="ua")
                nc.tensor.matmul(
                    up[:], ET[:, ds(t * P, P)], u_sb[:], start=True, stop=True
                )
                ups.append(up)
            for q in range(nt):
                t = t0 + q
                # q2c accumulation (single PSUM group spanning all blocks)
                nc.tensor.matmul(
                    hap[:], m_exp_r[:, ds(t, 1)], h_all[:, ds(t * D, D)],
                    start=(t == 0), stop=(t == NT - 1),
                    skip_group_check=True,
                )
            if tail_hook is not None:
                tail_hook()
            for q in range(nt):
                t = t0 + q
                up = ups[q]
                stg = stage.tile([P, 2 * D], f32, tag="stg")
                if latency:
                    # latency-critical early blocks: o2/o3 in parallel from
                    # PSUM on the two engines that can read it
                    nc.scalar.mul(stg[:, ds(0, D)], up[:], z_rec[:, ds(t, 1)])
                    nc.vector.scalar_tensor_tensor(
                        stg[:, ds(D, D)], up[:], z_rec[:, ds(t, 1)],
                        h_all[:, ds(t * D, D)],
                        op0=mybir.AluOpType.mult, op1=mybir.AluOpType.mult,
                    )
                else:
                    # throughput phase: o2 alternates ScalarE/DVE, GpSimd
                    # builds o3 from SBUF
                    if t % 2 == 0:
                        nc.scalar.mul(stg[:, ds(0, D)], up[:], z_rec[:, ds(t, 1)])
                    else:
                        nc.vector.tensor_scalar_mul(
                            stg[:, ds(0, D)], up[:], z_rec[:, ds(t, 1)]
                        )
                    nc.gpsimd.tensor_mul(
                        stg[:, ds(D, D)], stg[:, ds(0, D)],
                        h_all[:, ds(t * D, D)],
                    )
                nc.sync.dma_start(out_d[ds(t * P, P), ds(0, 2 * D)], stg[:])

        # Software-pipelined emission: warmup leads; block sizes ramp
        # [1,1,2,2,2] so the first store issues as early as possible; the
        # next block's transposes slot between a block's score matmuls and
        # its softmax tail to hide Exp/reduce latency.
        dve = nc.vector.tensor_copy
        sca = nc.scalar.copy
        BL = [(0, 2), (2, 2), (4, 2), (6, 2)]
        evs = {0: dve, 1: dve, 2: dve, 3: dve, 4: sca, 5: dve, 6: sca, 7: dve}

        mrow = consts.tile([P, 1], f32)
        rzq = consts.tile([1, 1], f32)
        ha_sum = consts.tile([1, D], f32)
        ha_row = consts.tile([1, D], f32r)
        zqp = [None]
        def q2c_tail():
            # emitted right after the final block's reduces: normalize and
            # broadcast h_a while that block's outputs are still in flight
            nc.vector.reduce_sum(mrow[:], m_exp_r[:], axis=AX)
            zq_ps = ps.tile([1, 1], f32, tag="acc", bufs=1)
            nc.tensor.matmul(zq_ps[:], mrow[:], ones_col[:], start=True, stop=True)
            nc.vector.reciprocal(rzq[:], zq_ps[:])
            nc.vector.tensor_copy(ha_sum[:], hap[:])
            nc.scalar.mul(ha_row[:], ha_sum[:], rzq[:])

        w_cols_r, uw, uwu = prep_weights()
        transpose_tile(0, dve)
        transpose_tile(1, dve)
        warm_mm(1)
        uwT = prep_uwT(uw)
        warm_mm(1)
        sp_prev = block_scores(0, 2, w_cols_r, uwT)
        warm_mm(1)
        for i in range(1, len(BL)):
            t0p, ntp = BL[i - 1]
            t0n, ntn = BL[i]
            for t in range(t0n, t0n + ntn):
                transpose_tile(t, evs[t])
            block_softmax_c2q(t0p, ntp, sp_prev, uwu, latency=(i == 1))
            sp_prev = block_scores(t0n, ntn, w_cols_r, uwT)
        t0l, ntl = BL[-1]
        block_softmax_c2q(t0l, ntl, sp_prev, uwu, tail_hook=q2c_tail)

        # ---- o4 = h * h_a ----
        bc = ps.tile([P, D], f32, tag="acc", bufs=1)
        nc.tensor.matmul(bc[:], ones_row_r[:], ha_row[:], start=True, stop=True)
        bc_sb = consts.tile([P, D], f32)  # gpsimd cannot read PSUM
        nc.scalar.copy(bc_sb[:], bc[:])

        for pair in range(NT // 2):
            t0 = 2 * pair
            o4 = stage.tile([P, 2 * D], f32, tag="o4")
            nc.gpsimd.tensor_mul(o4[:, ds(0, D)], h_all[:, ds(t0 * D, D)], bc_sb[:])
            nc.vector.tensor_mul(o4[:, ds(D, D)], h_all[:, ds((t0 + 1) * D, D)], bc[:])
            eng = nc.scalar if pair % 2 == 0 else nc.sync
            eng.dma_start(
                out_d[ds(t0 * P, 2 * P), ds(2 * D, D)].rearrange(
                    "(t p) d -> p t d", p=P
                ),
                o4[:].rearrange("p (t d) -> p t d", d=D),
            )

    nc.compile()
    return nc


def _get_nc():
    if "nc" not in _CACHE:
        _CACHE["nc"] = _build_program()
    return _CACHE["nc"]


def _assemble(h, results):
    """Host-side unshard: column 0 of the output is h verbatim."""
    out = np.empty((N_B, JX, 4 * D), dtype=np.float32)
    out[:, :, :D] = h
    for n in range(N_B):
        out[n, :, D:] = results[n]["out"]
    return out.reshape(N_B, M_B, JX, 4 * D)


def _ensure_axon_hooks_stub():
    # concourse imports antenv.axon_hooks when tracing is requested via env;
    # provide a no-op stub if the image lacks it so runs degrade gracefully.
    import sys
    import types

    try:
        import antenv.axon_hooks  # noqa: F401
    except ImportError:
        mod = types.ModuleType("antenv.axon_hooks")
        _hook = [None]
        mod.set_axon_ntff_profile_hook = lambda hook: _hook.__setitem__(0, hook)
        mod.get_axon_ntff_profile_hook = lambda: _hook[0]
        sys.modules["antenv.axon_hooks"] = mod


def kernel(h, u, alpha_w, alpha_b=None, **_unused):
    _ensure_axon_hooks_stub()
    from concourse.bass_utils import run_bass_kernel_spmd

    h = np.ascontiguousarray(np.asarray(h, dtype=np.float32)).reshape(N_B, JX, D)
    u = np.ascontiguousarray(np.asarray(u, dtype=np.float32)).reshape(N_B, JQ, D)
    alpha_w = np.ascontiguousarray(np.asarray(alpha_w, dtype=np.float32)).reshape(3 * D)

    nc = _get_nc()
    in_maps = [
        {"h": h[n], "u": u[n], "alpha_w": alpha_w} for n in range(N_B)
    ]
    res = run_bass_kernel_spmd(nc, in_maps, core_ids=list(range(N_B)))
    return _assemble(h, res.results)
